# revision 44
# baseline (speedup 1.0000x reference)
"""GQA (16 q-heads / 4 KV groups, S=4096, D=1024, causal) on 8 TRN2 NeuronCores.

Sharding: tensor-parallel over query heads - 2 q-heads + their KV group per
core. wq/wk/wv column-sharded, wo row-sharded; the 8 partial outputs are
summed on the host (no device collectives needed).

Per-core program (bf16 matmuls, f32 PSUM), ACT(exp)-bound design, ~220us:
  x is loaded chunk-major: one DMA per 512-token chunk, fully contiguous
    (8KB per partition per chunk); wide warm-up matmuls unthrottle the PE
    (HAM) and a dummy exp preloads the ACT table while DMAs fly.
  Projections per chunk n (2 matmul/c-step, accumulated in psum_w):
    set1 lhsT=[wq_h0|wq_h1] -> qT_sb, set2 lhsT=[wk|wv] -> kT (duplicated to
    both partition halves) + vT.  All psum->sbuf copies on DVE.
    v normal layout via PE-transpose (identity matmul) of vT 128-col tiles
    -> vaug [128, kt, 65] (col 64 = ones for the denominator row).
  Attention per q-chunk (512 q x 2 heads), per k-tile of 128 keys:
    qk: two row-tiled concurrent matmuls -> ps [128, 2, 512] (2 psum banks),
    emitted three tiles ahead of ctx (and 3 tiles pre-issued per chunk) so
    the scalar engine's next exp input is always ready.
    causal diagonal: DVE adds a -240 bias on the masked 128x128 block of the
    psum scores BEFORE exp (exp((s-240)/8) ~ 0), so ctx follows exp with no
    extra mask step.
    exp(s/8): ONE ACT instr over both heads (strided on diag strips).
    ctx: 2 matmuls (lhsT=vaug [128 keys, 65], row 64 = ones -> denominators)
    accumulated in ctx_ps [65, 2, 512].
  Out-proj + projections for chunk n+2 + the deferred reciprocal chain are
    emitted as deadline-tagged "fillers", interleaved 1-2 per attention
    k-tile (proj(n) force-drained before attention(n)), so the PE queue
    never makes the scalar engine (the bottleneck: ~147us of exp) wait.
    The reciprocal chain pops AFTER the proj fillers (keeps the single-lane
    DVE ops away from the congested chunk boundary) and out-proj is
    debt-scheduled at least one full chunk after its normalize, so its
    matmuls never stall on the rb DMA round trip; chunk 6's out-proj lands
    in the tail, soaking up the last reciprocal's latency.
  Normalize: DVE reciprocal of denom row -> bf16 -> DRAM -> partition-
    broadcast DMA -> rb [128, 1024]; ctxT2 [128, S] scaled in place (gpsimd).
  Out-proj per 128-row block: 2 matmuls (contraction 128, both heads at
    once), one DVE copy psum -> bf16 -> DMA to out partial [S, DIM] bf16.
  Tail: last chunk's recips are PE-broadcast (no DRAM latency), normalize +
    out-proj pipelined per 128-row block on the freed scores psum ring with
    copies split scalar/DVE; warm matmuls bridge the reciprocal latency.
Softmax uses no max-subtraction: s/8 ~ N(0,1) -> exp safe in f32.
"""

import numpy as np
import ml_dtypes
from collections import deque

BF16 = ml_dtypes.bfloat16

S = 4096
DIN = 1024
DIM = 1024
NH, NKV, HD = 16, 4, 64
NCORES = 8
QC = 512          # q chunk width per head
NQC = S // QC     # 8
NKT = S // 128    # 32 k tiles

_CACHE = {}


def _build_nc(debug=False):
    import concourse.bass as bass
    import concourse.mybir as mybir
    import concourse.tile as tile
    from concourse import bacc
    from concourse.masks import make_identity
    from concourse.tile_rust import add_dep_helper
    from contextlib import ExitStack

    fp32 = mybir.dt.float32
    bf16 = mybir.dt.bfloat16
    Exp = mybir.ActivationFunctionType.Exp

    nc = bacc.Bacc()
    # chunk-major x: xc[n, p, c*QC+s'] = x[n*QC+s', c*128+p]
    xc_d = nc.dram_tensor("xc", [NQC, 128, 8 * QC], bf16, kind="ExternalInput")
    # host pre-arranged [p, c, m]: row-contiguous DMA (128 descriptors)
    wqT_d = nc.dram_tensor("wqT", [128, 8 * 128], bf16, kind="ExternalInput")
    wkvT_d = nc.dram_tensor("wkvT", [128, 8 * 128], bf16, kind="ExternalInput")
    woT_d = nc.dram_tensor("woT", [128, DIM], bf16, kind="ExternalInput")
    mask_d = nc.dram_tensor("trimask", [128, 128], bf16, kind="ExternalInput")
    out_d = nc.dram_tensor("out", [S, DIM], bf16, kind="ExternalOutput")
    rec_d = nc.dram_tensor("recips_scratch", [NQC, 2 * QC], bf16)
    if debug:
        dbg_qT = nc.dram_tensor("dbg_qT", [128, NQC, QC], bf16, kind="ExternalOutput")
        dbg_kT = nc.dram_tensor("dbg_kT", [128, S], bf16, kind="ExternalOutput")
        dbg_vaug = nc.dram_tensor("dbg_vaug", [128, NKT, 65], bf16, kind="ExternalOutput")
        dbg_ctxT = nc.dram_tensor("dbg_ctxT", [128, S], bf16, kind="ExternalOutput")
        dbg_cps = nc.dram_tensor("dbg_cps", [128, 2, QC], mybir.dt.float32, kind="ExternalOutput")
        dbg_rb = nc.dram_tensor("dbg_rb", [128, 2, QC], bf16, kind="ExternalOutput")

    with ExitStack() as ctx:
        tc = ctx.enter_context(tile.TileContext(nc))
        singles = ctx.enter_context(tc.tile_pool(name="singles", bufs=1))
        pt_pool = ctx.enter_context(tc.tile_pool(name="pt", bufs=5))
        small = ctx.enter_context(tc.tile_pool(name="small", bufs=2))
        ostage = ctx.enter_context(tc.tile_pool(name="ostage", bufs=3))
        # PSUM budget (16KB/partition): ps_s 2x4KB + ps_w 1x4KB + ctx 1x4KB
        psum_s = ctx.enter_context(tc.tile_pool(name="psum_s", bufs=2, space="PSUM"))
        psum_w = ctx.enter_context(tc.tile_pool(name="psum_w", bufs=1, space="PSUM"))
        psum_ctx = ctx.enter_context(
            tc.tile_pool(name="psum_ctx", bufs=1, space="PSUM")
        )

        # ---- persistent SBUF tensors ----
        xT_sb = singles.tile([128, NQC, 8, QC], bf16, tag="xT")
        wqT_sb = singles.tile([128, 8, 128], bf16, tag="wqT")
        wkvT_sb = singles.tile([128, 8, 128], bf16, tag="wkvT")
        woT_sb = singles.tile([128, DIM], bf16, tag="woT")
        mask_sb = singles.tile([128, 128], bf16, tag="mask")
        ident_sb = singles.tile([64, 64], bf16, tag="ident")
        ones_sb = singles.tile([1, 64], bf16, tag="ones")
        qT_sb = singles.tile([128, NQC, QC], bf16, tag="qT")
        kT2_sb = singles.tile([128, S], bf16, tag="kT2")
        vT_sb = singles.tile([64, S], bf16, tag="vT")
        vaug_sb = singles.tile([128, NKT, 65], bf16, tag="vaug")
        ctxT2_sb = singles.tile([128, S], bf16, tag="ctxT2")

        # ---- input DMAs (chunk 0 halves + weights first) ----
        nc.sync.dma_start(out=xT_sb[:, 0, 0:4], in_=xc_d[0, :, 0:4 * QC])
        nc.sync.dma_start(
            out=wqT_sb, in_=wqT_d[:].rearrange("p (c m) -> p c m", c=8)
        )
        nc.sync.dma_start(
            out=wkvT_sb, in_=wkvT_d[:].rearrange("p (c m) -> p c m", c=8)
        )
        nc.sync.dma_start(out=xT_sb[:, 0, 4:8], in_=xc_d[0, :, 4 * QC:8 * QC])
        nc.sync.dma_start(out=xT_sb[:, 1], in_=xc_d[1])
        nc.sync.dma_start(out=mask_sb, in_=mask_d[:])
        nc.sync.dma_start(out=woT_sb, in_=woT_d[:])
        for n in range(2, NQC):
            nc.sync.dma_start(out=xT_sb[:, n], in_=xc_d[n])
        nc.vector.memset(vaug_sb[:, :, 64:65], 1.0)
        nc.vector.memset(ones_sb, 1.0)
        nc.vector.memset(ctxT2_sb[0:64, 0:QC], 0.0)
        make_identity(nc, ident_sb)
        # preload the exp table set while DMAs are in flight
        warm_sb = singles.tile([64, 64], bf16, tag="warm")
        nc.scalar.activation(warm_sb[0:1, 0:1], ident_sb[0:1, 0:1], Exp)
        # keep the PE busy until x arrives so HAM unthrottles before proj(0)
        warm_ps = psum_w.tile([64, QC], fp32, tag="ps_w", name="warm_ps")
        for _ in range(12):
            nc.tensor.matmul(
                warm_ps[0:64, :], ident_sb, ctxT2_sb[0:64, 0:QC],
                start=True, stop=True,
            )

        # ---------- building blocks ----------
        def proj_pieces(n):
            """Return list of filler callables for chunk n's projections."""
            hold = {}

            def mm(c):
                def f():
                    if c == 0:
                        hold["ps"] = psum_w.tile([128, 2, QC], fp32, tag="ps_w", name="ps_proj")
                    ps = hold["ps"]
                    xs = xT_sb[:, n, c, :]
                    nc.tensor.matmul(
                        ps[:, 0, :], wqT_sb[:, c, :], xs,
                        start=(c == 0), stop=(c == 7),
                    )
                    nc.tensor.matmul(
                        ps[:, 1, :], wkvT_sb[:, c, :], xs,
                        start=(c == 0), stop=(c == 7),
                    )
                return f

            cs = slice(n * QC, (n + 1) * QC)

            def cp_v():
                nc.vector.tensor_copy(vT_sb[:, cs], hold["ps"][64:128, 1, :])

            def cp_kl():
                if n < 2:
                    nc.scalar.copy(kT2_sb[0:64, cs], hold["ps"][0:64, 1, :])
                else:
                    nc.vector.tensor_copy(kT2_sb[0:64, cs], hold["ps"][0:64, 1, :])

            def cp_kh():
                nc.vector.tensor_copy(kT2_sb[64:128, cs], hold["ps"][0:64, 1, :])

            def cp_q():
                nc.vector.tensor_copy(qT_sb[:, n, :], hold["ps"][:, 0, :])

            def tr(t):
                def f():
                    if t == 0:
                        hold["tr"] = psum_w.tile([128, 4, 256], bf16, tag="ps_w", name="ps_tr")
                    kt = 4 * n + t
                    nc.tensor.transpose(
                        hold["tr"][:, t, 0:64],
                        vT_sb[:, kt * 128:(kt + 1) * 128],
                        ident_sb,
                    )
                return f

            def cp_vaug():
                nc.vector.tensor_copy(
                    vaug_sb[:, 4 * n:4 * n + 4, 0:64], hold["tr"][:, :, 0:64]
                )

            return (
                [mm(c) for c in range(8)]
                + [cp_kl, cp_kh, cp_q, cp_v]
                + [tr(t) for t in range(4)]
                + [cp_vaug]
            )

        def outproj_pieces(qc, pool=None, tag="ps_w", split_copy=False):
            """Fillers: per 128-row block an mm step and a copy+store step."""
            pool = pool or psum_w
            pieces = []
            hold = {}
            for rc in range(4 * qc, 4 * qc + 4):
                def mm(rc=rc):
                    ps_o = pool.tile([128, 2, QC], fp32, tag=tag, name="ps_o")
                    hold["ps"] = ps_o
                    lh = ctxT2_sb[:, rc * 128:(rc + 1) * 128]
                    for e in range(2):
                        nc.tensor.matmul(
                            ps_o[:, e, :], lh, woT_sb[:, e * QC:(e + 1) * QC],
                            start=True, stop=True,
                        )

                def st(rc=rc):
                    ot = ostage.tile([128, DIM], bf16, tag="ot")
                    if split_copy:
                        nc.scalar.copy(ot[:, 0:QC], hold["ps"][:, 0, :])
                        nc.vector.tensor_copy(ot[:, QC:DIM], hold["ps"][:, 1, :])
                    else:
                        nc.vector.tensor_copy(ot, hold["ps"][:, :, :])
                    nc.sync.dma_start(
                        out=out_d[rc * 128:(rc + 1) * 128, :], in_=ot
                    )

                pieces += [mm, st]
            return pieces

        def normalize(qc, rb):
            cs = slice(qc * QC, (qc + 1) * QC)
            nc.gpsimd.tensor_mul(
                ctxT2_sb[0:64, cs], ctxT2_sb[0:64, cs], rb[0:64, 0, :]
            )
            nc.gpsimd.tensor_mul(
                ctxT2_sb[64:128, cs], ctxT2_sb[64:128, cs], rb[64:128, 1, :]
            )

        fillers = deque()

        def pop_fillers(k, warm=False):
            for _ in range(k):
                if fillers:
                    fillers.popleft()[1]()
                elif warm:
                    return

        def qk_exp(qc, kt):
            r = kt - 4 * qc
            off = 128 * r if r >= 1 else 0
            ps = psum_s.tile([128, 2, QC], fp32, tag="ps_s", name="ps")
            pt = pt_pool.tile([128, 2, QC], bf16, tag="pt", name="pt")
            ktl = kT2_sb[0:64, kt * 128:(kt + 1) * 128]
            kth = kT2_sb[64:128, kt * 128:(kt + 1) * 128]
            nc.tensor.matmul(
                ps[:, 0, off:QC], ktl, qT_sb[0:64, qc, off:QC],
                start=True, stop=True,
            )
            nc.tensor.matmul(
                ps[:, 1, off:QC], kth, qT_sb[64:128, qc, off:QC],
                start=True, stop=True, tile_position=(64, 0),
            )
            if r >= 0:  # causal diagonal block: add -240 bias pre-exp
                for h in range(2):
                    nc.vector.tensor_add(
                        ps[:, h, off:off + 128],
                        ps[:, h, off:off + 128],
                        mask_sb,
                    )
            nc.scalar.activation(
                pt[:, :, off:QC], ps[:, :, off:QC], Exp, scale=0.125
            )
            return pt

        carry = {}  # scores of the next chunk's first tile, pre-issued

        def attention(qc):
            nkt = 4 * qc + 4
            # scores always run three tiles ahead of ctx so the scalar
            # engine's next exp input is ready before it finishes the current
            pts = {}
            if 0 in carry:
                pts[0] = carry.pop(0)
            for kt in range(min(3, nkt)):
                if kt not in pts:
                    pts[kt] = qk_exp(qc, kt)
            ctx_ps = psum_ctx.tile([128, 2, QC], fp32, tag="ps_ctx")
            for kt in range(nkt):
                r = kt - 4 * qc
                off = 128 * r if r >= 1 else 0
                if kt + 3 < nkt:
                    pts[kt + 3] = qk_exp(qc, kt + 3)
                elif (kt == nkt - 1 and qc + 1 < NQC
                        and not any(t <= qc for t, _ in fillers)):
                    # bridge the boundary: next chunk's first scores go out
                    # now so ACT rolls into it with no bubble (safe only once
                    # proj(qc+1) is fully emitted)
                    carry[0] = qk_exp(qc + 1, 0)
                pt = pts.pop(kt)
                va = vaug_sb[:, kt, :]
                nc.tensor.matmul(
                    ctx_ps[0:65, 0, off:QC], va, pt[:, 0, off:QC],
                    start=(kt == 0), stop=(kt == nkt - 1),
                )
                nc.tensor.matmul(
                    ctx_ps[0:65, 1, off:QC], va, pt[:, 1, off:QC],
                    start=(kt == 0), stop=(kt == nkt - 1),
                )
                pop_fillers(2 if (kt < 6 or len(fillers) > 32) else 1,
                            warm=(qc >= 5))
            return ctx_ps

        def finish_ctx(qc, ctx_ps, last=False):
            """Copy raw ctx to stacked sbuf and stage the denominator row;
            the reciprocal chain is deferred into the next chunk's fillers."""
            cs = slice(qc * QC, (qc + 1) * QC)
            den = small.tile([1, 2, QC], fp32, tag="den")
            nc.vector.tensor_copy(den, ctx_ps[64:65, :, :])
            if last:  # scalar engine is idle in the tail
                nc.scalar.copy(ctxT2_sb[0:64, cs], ctx_ps[0:64, 0, :])
                nc.scalar.copy(ctxT2_sb[64:128, cs], ctx_ps[0:64, 1, :])
            else:
                nc.vector.tensor_copy(ctxT2_sb[0:64, cs], ctx_ps[0:64, 0, :])
                nc.vector.tensor_copy(ctxT2_sb[64:128, cs], ctx_ps[0:64, 1, :])
            return den

        def make_rb(den, last=False, row=0):
            """Reciprocal of the staged denominators + partition broadcast
            (DRAM round trip); for the tail, return recb for a PE broadcast."""
            nc.vector.reciprocal_approx_fast(den, den)
            recb = small.tile([1, 2, QC], bf16, tag="recb")
            nc.vector.tensor_copy(recb, den)
            if last:
                return recb
            rb = small.tile([128, 2, QC], bf16, tag="rb")
            w1 = nc.sync.dma_start(out=rec_d[row:row + 1, :], in_=recb)
            srcp = rec_d[row, :]
            r1 = nc.sync.dma_start(
                out=rb,
                in_=bass.AP(tensor=srcp.tensor, offset=srcp.offset,
                            ap=[[0, 128]] + list(srcp.ap)),
            )
            add_dep_helper(r1.ins, w1.ins, reason="recips dram RAW")
            return rb

        # ---- main pipeline ----
        # proj(0) runs eagerly; proj(1) drains into attention(0);
        # proj(n+2) + outproj(n-1) interleave into attention(n), a couple
        # of filler pieces per k-tile.
        for f in proj_pieces(0):
            f()
        fillers.extend((0, f) for f in proj_pieces(1))
        prev = None  # (qc, den) awaiting recip+normalize+outproj
        oproj_debt = deque()  # chunks normalized but not yet projected out
        hold_rb = {}
        NODL = 99  # no deadline

        def force_due(qc):
            due = -1
            for i, (t, _) in enumerate(fillers):
                if t <= qc:
                    due = i
            for _ in range(due + 1):
                fillers.popleft()[1]()

        for qc in range(NQC):
            if qc + 2 < NQC:
                # proj(qc+2) must fully drain by the end of attention(qc+1)
                fillers.extend((qc + 1, f) for f in proj_pieces(qc + 2))
            if prev is not None:
                pq, den = prev
                oproj_debt.append(pq)

                def mk(pq=pq, den=den):
                    hold_rb[pq] = make_rb(den, row=pq)
                fillers.append((NODL, mk))
                fillers.append(
                    (NODL, lambda pq=pq: normalize(pq, hold_rb.pop(pq)))
                )
            quota = 1 if qc < 6 else 2
            for _ in range(quota):
                # only chunks normalized in an EARLIER iteration (no stall
                # on the fresh rb round trip)
                if oproj_debt and (prev is None or oproj_debt[0] != prev[0]):
                    fillers.extend(
                        (NODL, f) for f in outproj_pieces(oproj_debt.popleft())
                    )
            ctx_ps = attention(qc)
            force_due(qc)  # next chunk's proj ahead of the DVE den staging
            den = finish_ctx(qc, ctx_ps, last=(qc == NQC - 1))
            prev = (qc, den)
        while fillers:
            fillers.popleft()[1]()
        for pq2 in oproj_debt:  # any outproj still owed (e.g. chunk 6)
            for f in outproj_pieces(pq2):
                f()
        # tail: keep the PE warm while the reciprocal chain runs, then
        # PE-broadcast the recips (no DRAM latency) and pipeline
        # normalize+outproj per 128-row block on the free scores psum ring
        pq, den = prev
        recb = make_rb(den, last=True)
        warm2 = psum_s.tile([64, QC], fp32, tag="ps_s", name="warm2")
        for _ in range(14):
            nc.tensor.matmul(warm2[0:64, :], ident_sb, ctxT2_sb[0:64, 0:QC],
                             start=True, stop=True)
        bps = psum_w.tile([128, 2, QC], fp32, tag="ps_w", name="bps")
        nc.tensor.matmul(bps[0:64, 0, :], ones_sb, recb[0:1, 0, :],
                         start=True, stop=True)
        nc.tensor.matmul(bps[0:64, 1, :], ones_sb, recb[0:1, 1, :],
                         start=True, stop=True)
        rbs = small.tile([128, 2, QC], bf16, tag="rb")
        for i, rc in enumerate(range(4 * pq, 4 * pq + 4)):
            cb = slice(rc * 128, (rc + 1) * 128)
            qb = slice(i * 128, (i + 1) * 128)
            nc.vector.tensor_copy(rbs[0:64, 0, qb], bps[0:64, 0, qb])
            nc.vector.tensor_copy(rbs[64:128, 1, qb], bps[0:64, 1, qb])
            nc.vector.tensor_mul(
                ctxT2_sb[0:64, cb], ctxT2_sb[0:64, cb], rbs[0:64, 0, qb]
            )
            nc.vector.tensor_mul(
                ctxT2_sb[64:128, cb], ctxT2_sb[64:128, cb], rbs[64:128, 1, qb]
            )
            ps_o = psum_s.tile([128, 2, QC], fp32, tag="ps_s", name="ps_ot")
            lh = ctxT2_sb[:, cb]
            for e in range(2):
                nc.tensor.matmul(
                    ps_o[:, e, :], lh, woT_sb[:, e * QC:(e + 1) * QC],
                    start=True, stop=True,
                )
            ot = ostage.tile([128, DIM], bf16, tag="ot")
            nc.scalar.copy(ot[:, 0:QC], ps_o[:, 0, :])
            nc.vector.tensor_copy(ot[:, QC:DIM], ps_o[:, 1, :])
            nc.sync.dma_start(out=out_d[rc * 128:(rc + 1) * 128, :], in_=ot)

        if debug:
            nc.sync.dma_start(out=dbg_qT[:], in_=qT_sb)
            nc.sync.dma_start(out=dbg_kT[:], in_=kT2_sb)
            nc.sync.dma_start(out=dbg_vaug[:], in_=vaug_sb)
            nc.sync.dma_start(out=dbg_ctxT[:], in_=ctxT2_sb)

    nc.compile()
    return nc


def _get_nc():
    if "nc" not in _CACHE:
        _CACHE["nc"] = _build_nc()
    return _CACHE["nc"]


def _prep_inputs(x, wq, wk, wv, wo):
    GS = NH // NKV
    x2 = np.asarray(x, np.float32).reshape(S, DIN)
    xT = np.ascontiguousarray(x2.T)
    # chunk-major: xc[n, p, c, s'] = xT[c*128+p, n*QC+s']
    xc = np.ascontiguousarray(
        xT.reshape(8, 128, NQC, QC).transpose(2, 1, 0, 3).reshape(NQC, 128, 8 * QC)
    ).astype(BF16)
    # additive causal bias for the diagonal 128x128 block: 0 where q >= k,
    # -240 where masked (exp((s-240)/8) ~ 0)
    tri = np.where(
        np.arange(128)[None, :] >= np.arange(128)[:, None], 0.0, -240.0
    ).astype(BF16)
    in_maps = []
    for c in range(NCORES):
        h0 = 2 * c
        g = h0 // GS
        wq_c = np.asarray(wq, np.float32)[h0 * HD:(h0 + 2) * HD, :]
        wkv_c = np.concatenate(
            [
                np.asarray(wk, np.float32)[g * HD:(g + 1) * HD, :],
                np.asarray(wv, np.float32)[g * HD:(g + 1) * HD, :],
            ],
            axis=0,
        )
        woT_c = np.asarray(wo, np.float32)[:, h0 * HD:(h0 + 2) * HD].T

        def prearrange(wT):  # [1024, 128] -> [p, c*m] = [128, 1024]
            return np.ascontiguousarray(
                wT.reshape(8, 128, 128).transpose(1, 0, 2).reshape(128, 1024)
            )

        in_maps.append(
            {
                "xc": xc,
                "wqT": prearrange(np.ascontiguousarray(wq_c.T)).astype(BF16),
                "wkvT": prearrange(np.ascontiguousarray(wkv_c.T)).astype(BF16),
                "woT": np.ascontiguousarray(woT_c).astype(BF16),
                "trimask": tri,
            }
        )
    return in_maps


def _run(in_maps, trace=False):
    import sys
    if "/opt/trn_rl_repo" not in sys.path:
        sys.path.insert(0, "/opt/trn_rl_repo")
    from concourse.bass_utils import run_bass_kernel_spmd

    nc = _get_nc()
    res = run_bass_kernel_spmd(nc, in_maps, list(range(NCORES)), trace=trace)
    return res


def kernel(x, wq, wk, wv, wo):
    in_maps = _prep_inputs(x, wq, wk, wv, wo)
    res = _run(in_maps)
    parts = np.stack(
        [np.asarray(r["out"], np.float32) for r in res.results]
    )
    out = parts.sum(axis=0, dtype=np.float64).astype(np.float32)
    return out.reshape(1, S, DIM)


# revision 45
# speedup vs baseline: 1.0091x; 1.0091x over previous
"""GQA (16 q-heads / 4 KV groups, S=4096, D=1024, causal) on 8 TRN2 NeuronCores.

Sharding: tensor-parallel over query heads - 2 q-heads + their KV group per
core. wq/wk/wv column-sharded, wo row-sharded; the 8 partial outputs are
summed on the host (no device collectives needed).

Per-core program (bf16 matmuls, f32 PSUM), ACT(exp)-bound design, ~220us:
  x is loaded chunk-major: one DMA per 512-token chunk, fully contiguous
    (8KB per partition per chunk); wide warm-up matmuls unthrottle the PE
    (HAM) and a dummy exp preloads the ACT table while DMAs fly.
  Projections per chunk n (2 matmul/c-step, accumulated in psum_w):
    set1 lhsT=[wq_h0|wq_h1] -> qT_sb, set2 lhsT=[wk|wv] -> kT (duplicated to
    both partition halves) + vT.  All psum->sbuf copies on DVE.
    v normal layout via PE-transpose (identity matmul) of vT 128-col tiles
    -> vaug [128, kt, 65] (col 64 = ones for the denominator row).
  Attention per q-chunk (512 q x 2 heads), per k-tile of 128 keys:
    qk: two row-tiled concurrent matmuls -> ps [128, 2, 512] (2 psum banks),
    emitted three tiles ahead of ctx (and 3 tiles pre-issued per chunk) so
    the scalar engine's next exp input is always ready.
    causal diagonal: DVE adds a -240 bias on the masked 128x128 block of the
    psum scores BEFORE exp (exp((s-240)/8) ~ 0), so ctx follows exp with no
    extra mask step.
    exp(s/8): ONE ACT instr over both heads (strided on diag strips).
    ctx: 2 matmuls (lhsT=vaug [128 keys, 65], row 64 = ones -> denominators)
    accumulated in ctx_ps [65, 2, 512].
  Out-proj + projections for chunk n+2 + the deferred reciprocal chain are
    emitted as deadline-tagged "fillers", interleaved 1-2 per attention
    k-tile (proj(n) force-drained before attention(n)), so the PE queue
    never makes the scalar engine (the bottleneck: ~147us of exp) wait.
    The reciprocal chain pops AFTER the proj fillers (keeps the single-lane
    DVE ops away from the congested chunk boundary) and out-proj is
    debt-scheduled at least one full chunk after its normalize, so its
    matmuls never stall on the rb DMA round trip; chunk 6's out-proj lands
    in the tail, soaking up the last reciprocal's latency.
  Normalize: DVE reciprocal of denom row -> bf16 -> DRAM -> partition-
    broadcast DMA -> rb [128, 1024]; ctxT2 [128, S] scaled in place (gpsimd).
  Out-proj per 128-row block: 2 matmuls (contraction 128, both heads at
    once), one DVE copy psum -> bf16 -> DMA to out partial [S, DIM] bf16.
  Tail: last chunk's recips are PE-broadcast (no DRAM latency), normalize +
    out-proj pipelined per 128-row block on the freed scores psum ring with
    copies split scalar/DVE; warm matmuls bridge the reciprocal latency.
Softmax uses no max-subtraction: s/8 ~ N(0,1) -> exp safe in f32.
"""

import numpy as np
import ml_dtypes
from collections import deque

BF16 = ml_dtypes.bfloat16

S = 4096
DIN = 1024
DIM = 1024
NH, NKV, HD = 16, 4, 64
NCORES = 8
QC = 512          # q chunk width per head
NQC = S // QC     # 8
NKT = S // 128    # 32 k tiles

_CACHE = {}


def _build_nc(debug=False):
    import concourse.bass as bass
    import concourse.mybir as mybir
    import concourse.tile as tile
    from concourse import bacc
    from concourse.masks import make_identity
    from concourse.tile_rust import add_dep_helper
    from contextlib import ExitStack

    fp32 = mybir.dt.float32
    bf16 = mybir.dt.bfloat16
    Exp = mybir.ActivationFunctionType.Exp

    nc = bacc.Bacc()
    # chunk-major x: xc[n, p, c*QC+s'] = x[n*QC+s', c*128+p]
    xc_d = nc.dram_tensor("xc", [NQC, 128, 8 * QC], bf16, kind="ExternalInput")
    # host pre-arranged [p, c, m]: row-contiguous DMA (128 descriptors)
    wqT_d = nc.dram_tensor("wqT", [128, 8 * 128], bf16, kind="ExternalInput")
    wkvT_d = nc.dram_tensor("wkvT", [128, 8 * 128], bf16, kind="ExternalInput")
    woT_d = nc.dram_tensor("woT", [128, DIM], bf16, kind="ExternalInput")
    mask_d = nc.dram_tensor("trimask", [128, 128], bf16, kind="ExternalInput")
    out_d = nc.dram_tensor("out", [S, DIM], bf16, kind="ExternalOutput")
    rec_d = nc.dram_tensor("recips_scratch", [NQC, 2 * QC], bf16)
    if debug:
        dbg_qT = nc.dram_tensor("dbg_qT", [128, NQC, QC], bf16, kind="ExternalOutput")
        dbg_kT = nc.dram_tensor("dbg_kT", [128, S], bf16, kind="ExternalOutput")
        dbg_vaug = nc.dram_tensor("dbg_vaug", [128, NKT, 65], bf16, kind="ExternalOutput")
        dbg_ctxT = nc.dram_tensor("dbg_ctxT", [128, S], bf16, kind="ExternalOutput")
        dbg_cps = nc.dram_tensor("dbg_cps", [128, 2, QC], mybir.dt.float32, kind="ExternalOutput")
        dbg_rb = nc.dram_tensor("dbg_rb", [128, 2, QC], bf16, kind="ExternalOutput")

    with ExitStack() as ctx:
        tc = ctx.enter_context(tile.TileContext(nc))
        singles = ctx.enter_context(tc.tile_pool(name="singles", bufs=1))
        pt_pool = ctx.enter_context(tc.tile_pool(name="pt", bufs=6))
        small = ctx.enter_context(tc.tile_pool(name="small", bufs=3))
        ostage = ctx.enter_context(tc.tile_pool(name="ostage", bufs=4))
        # PSUM budget (16KB/partition): ps_s 2x4KB + ps_w 1x4KB + ctx 1x4KB
        psum_s = ctx.enter_context(tc.tile_pool(name="psum_s", bufs=2, space="PSUM"))
        psum_w = ctx.enter_context(tc.tile_pool(name="psum_w", bufs=1, space="PSUM"))
        psum_ctx = ctx.enter_context(
            tc.tile_pool(name="psum_ctx", bufs=1, space="PSUM")
        )

        # ---- persistent SBUF tensors ----
        xT_sb = singles.tile([128, NQC, 8, QC], bf16, tag="xT")
        wqT_sb = singles.tile([128, 8, 128], bf16, tag="wqT")
        wkvT_sb = singles.tile([128, 8, 128], bf16, tag="wkvT")
        woT_sb = singles.tile([128, DIM], bf16, tag="woT")
        mask_sb = singles.tile([128, 128], bf16, tag="mask")
        ident_sb = singles.tile([64, 64], bf16, tag="ident")
        ones_sb = singles.tile([1, 64], bf16, tag="ones")
        qT_sb = singles.tile([128, NQC, QC], bf16, tag="qT")
        kT2_sb = singles.tile([128, S], bf16, tag="kT2")
        vT_sb = singles.tile([64, S], bf16, tag="vT")
        vaug_sb = singles.tile([128, NKT, 65], bf16, tag="vaug")
        ctxT2_sb = singles.tile([128, S], bf16, tag="ctxT2")

        # ---- input DMAs (chunk 0 halves + weights first) ----
        nc.sync.dma_start(out=xT_sb[:, 0, 0:4], in_=xc_d[0, :, 0:4 * QC])
        nc.sync.dma_start(
            out=wqT_sb, in_=wqT_d[:].rearrange("p (c m) -> p c m", c=8)
        )
        nc.sync.dma_start(
            out=wkvT_sb, in_=wkvT_d[:].rearrange("p (c m) -> p c m", c=8)
        )
        nc.sync.dma_start(out=xT_sb[:, 0, 4:8], in_=xc_d[0, :, 4 * QC:8 * QC])
        nc.sync.dma_start(out=xT_sb[:, 1], in_=xc_d[1])
        nc.sync.dma_start(out=mask_sb, in_=mask_d[:])
        nc.sync.dma_start(out=woT_sb, in_=woT_d[:])
        for n in range(2, NQC):
            nc.sync.dma_start(out=xT_sb[:, n], in_=xc_d[n])
        nc.vector.memset(vaug_sb[:, :, 64:65], 1.0)
        nc.vector.memset(ones_sb, 1.0)
        nc.vector.memset(ctxT2_sb[0:64, 0:QC], 0.0)
        make_identity(nc, ident_sb)
        # preload the exp table set while DMAs are in flight
        warm_sb = singles.tile([64, 64], bf16, tag="warm")
        nc.scalar.activation(warm_sb[0:1, 0:1], ident_sb[0:1, 0:1], Exp)
        # keep the PE busy until x arrives so HAM unthrottles before proj(0)
        warm_ps = psum_w.tile([64, QC], fp32, tag="ps_w", name="warm_ps")
        for _ in range(12):
            nc.tensor.matmul(
                warm_ps[0:64, :], ident_sb, ctxT2_sb[0:64, 0:QC],
                start=True, stop=True,
            )

        # ---------- building blocks ----------
        def proj_pieces(n):
            """Return list of filler callables for chunk n's projections."""
            hold = {}

            def mm(c):
                def f():
                    if c == 0:
                        hold["ps"] = psum_w.tile([128, 2, QC], fp32, tag="ps_w", name="ps_proj")
                    ps = hold["ps"]
                    xs = xT_sb[:, n, c, :]
                    nc.tensor.matmul(
                        ps[:, 0, :], wqT_sb[:, c, :], xs,
                        start=(c == 0), stop=(c == 7),
                    )
                    nc.tensor.matmul(
                        ps[:, 1, :], wkvT_sb[:, c, :], xs,
                        start=(c == 0), stop=(c == 7),
                    )
                return f

            cs = slice(n * QC, (n + 1) * QC)

            def cp_v():
                nc.vector.tensor_copy(vT_sb[:, cs], hold["ps"][64:128, 1, :])

            def cp_kl():
                if n < 2:
                    nc.scalar.copy(kT2_sb[0:64, cs], hold["ps"][0:64, 1, :])
                else:
                    nc.vector.tensor_copy(kT2_sb[0:64, cs], hold["ps"][0:64, 1, :])

            def cp_kh():
                nc.vector.tensor_copy(kT2_sb[64:128, cs], hold["ps"][0:64, 1, :])

            def cp_q():
                nc.vector.tensor_copy(qT_sb[:, n, :], hold["ps"][:, 0, :])

            def tr(t):
                def f():
                    if t == 0:
                        hold["tr"] = psum_w.tile([128, 4, 256], bf16, tag="ps_w", name="ps_tr")
                    kt = 4 * n + t
                    nc.tensor.transpose(
                        hold["tr"][:, t, 0:64],
                        vT_sb[:, kt * 128:(kt + 1) * 128],
                        ident_sb,
                    )
                return f

            def cp_vaug():
                nc.vector.tensor_copy(
                    vaug_sb[:, 4 * n:4 * n + 4, 0:64], hold["tr"][:, :, 0:64]
                )

            return (
                [mm(c) for c in range(8)]
                + [cp_kl, cp_kh, cp_q, cp_v]
                + [tr(t) for t in range(4)]
                + [cp_vaug]
            )

        def outproj_pieces(qc, pool=None, tag="ps_w", split_copy=False):
            """Fillers: per 128-row block an mm step and a copy+store step."""
            pool = pool or psum_w
            pieces = []
            hold = {}
            for rc in range(4 * qc, 4 * qc + 4):
                def mm(rc=rc):
                    ps_o = pool.tile([128, 2, QC], fp32, tag=tag, name="ps_o")
                    hold["ps"] = ps_o
                    lh = ctxT2_sb[:, rc * 128:(rc + 1) * 128]
                    for e in range(2):
                        nc.tensor.matmul(
                            ps_o[:, e, :], lh, woT_sb[:, e * QC:(e + 1) * QC],
                            start=True, stop=True,
                        )

                def st(rc=rc):
                    ot = ostage.tile([128, DIM], bf16, tag="ot")
                    if split_copy:
                        nc.scalar.copy(ot[:, 0:QC], hold["ps"][:, 0, :])
                        nc.vector.tensor_copy(ot[:, QC:DIM], hold["ps"][:, 1, :])
                    else:
                        nc.vector.tensor_copy(ot, hold["ps"][:, :, :])
                    nc.sync.dma_start(
                        out=out_d[rc * 128:(rc + 1) * 128, :], in_=ot
                    )

                pieces += [mm, st]
            return pieces

        def normalize(qc, rb):
            cs = slice(qc * QC, (qc + 1) * QC)
            nc.gpsimd.tensor_mul(
                ctxT2_sb[0:64, cs], ctxT2_sb[0:64, cs], rb[0:64, 0, :]
            )
            nc.gpsimd.tensor_mul(
                ctxT2_sb[64:128, cs], ctxT2_sb[64:128, cs], rb[64:128, 1, :]
            )

        fillers = deque()

        def pop_fillers(k, warm=False):
            for _ in range(k):
                if fillers:
                    fillers.popleft()[1]()
                elif warm:
                    return

        def attention(qc):
            nkt = 4 * qc + 4

            def qk_exp(kt):
                r = kt - 4 * qc
                off = 128 * r if r >= 1 else 0
                ps = psum_s.tile([128, 2, QC], fp32, tag="ps_s", name="ps")
                pt = pt_pool.tile([128, 2, QC], bf16, tag="pt", name="pt")
                ktl = kT2_sb[0:64, kt * 128:(kt + 1) * 128]
                kth = kT2_sb[64:128, kt * 128:(kt + 1) * 128]
                nc.tensor.matmul(
                    ps[:, 0, off:QC], ktl, qT_sb[0:64, qc, off:QC],
                    start=True, stop=True,
                )
                nc.tensor.matmul(
                    ps[:, 1, off:QC], kth, qT_sb[64:128, qc, off:QC],
                    start=True, stop=True, tile_position=(64, 0),
                )
                if r >= 0:  # causal diagonal block: add -240 bias pre-exp
                    for h in range(2):
                        nc.vector.tensor_add(
                            ps[:, h, off:off + 128],
                            ps[:, h, off:off + 128],
                            mask_sb,
                        )
                nc.scalar.activation(
                    pt[:, :, off:QC], ps[:, :, off:QC], Exp, scale=0.125
                )
                return pt

            # scores always run two tiles ahead of ctx so the scalar
            # engine's next exp input is ready before it finishes the current
            pts = {kt: qk_exp(kt) for kt in range(min(3, nkt))}
            ctx_ps = psum_ctx.tile([128, 2, QC], fp32, tag="ps_ctx")
            for kt in range(nkt):
                r = kt - 4 * qc
                off = 128 * r if r >= 1 else 0
                if kt + 3 < nkt:
                    pts[kt + 3] = qk_exp(kt + 3)
                pt = pts.pop(kt)
                va = vaug_sb[:, kt, :]
                nc.tensor.matmul(
                    ctx_ps[0:65, 0, off:QC], va, pt[:, 0, off:QC],
                    start=(kt == 0), stop=(kt == nkt - 1),
                )
                nc.tensor.matmul(
                    ctx_ps[0:65, 1, off:QC], va, pt[:, 1, off:QC],
                    start=(kt == 0), stop=(kt == nkt - 1),
                )
                pop_fillers(2 if (kt < 6 or len(fillers) > 32) else 1,
                            warm=(qc >= 5))
            return ctx_ps

        def finish_ctx(qc, ctx_ps, last=False):
            """Copy raw ctx to stacked sbuf and stage the denominator row;
            the reciprocal chain is deferred into the next chunk's fillers."""
            cs = slice(qc * QC, (qc + 1) * QC)
            den = small.tile([1, 2, QC], fp32, tag="den")
            nc.vector.tensor_copy(den, ctx_ps[64:65, :, :])
            if last:  # scalar engine is idle in the tail
                nc.scalar.copy(ctxT2_sb[0:64, cs], ctx_ps[0:64, 0, :])
                nc.scalar.copy(ctxT2_sb[64:128, cs], ctx_ps[0:64, 1, :])
            else:
                nc.vector.tensor_copy(ctxT2_sb[0:64, cs], ctx_ps[0:64, 0, :])
                nc.vector.tensor_copy(ctxT2_sb[64:128, cs], ctx_ps[0:64, 1, :])
            return den

        def make_rb(den, last=False, row=0):
            """Reciprocal of the staged denominators + partition broadcast
            (DRAM round trip); for the tail, return recb for a PE broadcast."""
            nc.vector.reciprocal_approx_fast(den, den)
            recb = small.tile([1, 2, QC], bf16, tag="recb")
            nc.vector.tensor_copy(recb, den)
            if last:
                return recb
            rb = small.tile([128, 2, QC], bf16, tag="rb")
            w1 = nc.sync.dma_start(out=rec_d[row:row + 1, :], in_=recb)
            srcp = rec_d[row, :]
            r1 = nc.sync.dma_start(
                out=rb,
                in_=bass.AP(tensor=srcp.tensor, offset=srcp.offset,
                            ap=[[0, 128]] + list(srcp.ap)),
            )
            add_dep_helper(r1.ins, w1.ins, reason="recips dram RAW")
            return rb

        # ---- main pipeline ----
        # proj(0) runs eagerly; proj(1) drains into attention(0);
        # proj(n+2) + outproj(n-1) interleave into attention(n), a couple
        # of filler pieces per k-tile.
        for f in proj_pieces(0):
            f()
        fillers.extend((0, f) for f in proj_pieces(1))
        prev = None  # (qc, den) awaiting recip+normalize+outproj
        oproj_debt = deque()  # chunks normalized but not yet projected out
        hold_rb = {}
        NODL = 99  # no deadline

        def force_due(qc):
            due = -1
            for i, (t, _) in enumerate(fillers):
                if t <= qc:
                    due = i
            for _ in range(due + 1):
                fillers.popleft()[1]()

        for qc in range(NQC):
            if qc + 2 < NQC:
                # proj(qc+2) must fully drain by the end of attention(qc+1)
                fillers.extend((qc + 1, f) for f in proj_pieces(qc + 2))
            if prev is not None:
                pq, den = prev
                oproj_debt.append(pq)

                def mk(pq=pq, den=den):
                    hold_rb[pq] = make_rb(den, row=pq)
                fillers.append((NODL, mk))
                fillers.append(
                    (NODL, lambda pq=pq: normalize(pq, hold_rb.pop(pq)))
                )
            quota = 1 if qc < 6 else 2
            for _ in range(quota):
                # only chunks normalized in an EARLIER iteration (no stall
                # on the fresh rb round trip)
                if oproj_debt and (prev is None or oproj_debt[0] != prev[0]):
                    fillers.extend(
                        (NODL, f) for f in outproj_pieces(oproj_debt.popleft())
                    )
            ctx_ps = attention(qc)
            force_due(qc)  # next chunk's proj ahead of the DVE den staging
            den = finish_ctx(qc, ctx_ps, last=(qc == NQC - 1))
            prev = (qc, den)
        while fillers:
            fillers.popleft()[1]()
        for pq2 in oproj_debt:  # any outproj still owed (e.g. chunk 6)
            for f in outproj_pieces(pq2):
                f()
        # tail: keep the PE warm while the reciprocal chain runs, then
        # PE-broadcast the recips (no DRAM latency) and pipeline
        # normalize+outproj per 128-row block on the free scores psum ring
        pq, den = prev
        recb = make_rb(den, last=True)
        warm2 = psum_s.tile([64, QC], fp32, tag="ps_s", name="warm2")
        for _ in range(14):
            nc.tensor.matmul(warm2[0:64, :], ident_sb, ctxT2_sb[0:64, 0:QC],
                             start=True, stop=True)
        bps = psum_w.tile([128, 2, QC], fp32, tag="ps_w", name="bps")
        nc.tensor.matmul(bps[0:64, 0, :], ones_sb, recb[0:1, 0, :],
                         start=True, stop=True)
        nc.tensor.matmul(bps[0:64, 1, :], ones_sb, recb[0:1, 1, :],
                         start=True, stop=True)
        rbs = small.tile([128, 2, QC], bf16, tag="rb")
        for i, rc in enumerate(range(4 * pq, 4 * pq + 4)):
            cb = slice(rc * 128, (rc + 1) * 128)
            qb = slice(i * 128, (i + 1) * 128)
            nc.vector.tensor_copy(rbs[0:64, 0, qb], bps[0:64, 0, qb])
            nc.vector.tensor_copy(rbs[64:128, 1, qb], bps[0:64, 1, qb])
            nc.vector.tensor_mul(
                ctxT2_sb[0:64, cb], ctxT2_sb[0:64, cb], rbs[0:64, 0, qb]
            )
            nc.vector.tensor_mul(
                ctxT2_sb[64:128, cb], ctxT2_sb[64:128, cb], rbs[64:128, 1, qb]
            )
            ps_o = psum_s.tile([128, 2, QC], fp32, tag="ps_s", name="ps_ot")
            lh = ctxT2_sb[:, cb]
            for e in range(2):
                nc.tensor.matmul(
                    ps_o[:, e, :], lh, woT_sb[:, e * QC:(e + 1) * QC],
                    start=True, stop=True,
                )
            ot = ostage.tile([128, DIM], bf16, tag="ot")
            nc.scalar.copy(ot[:, 0:QC], ps_o[:, 0, :])
            nc.vector.tensor_copy(ot[:, QC:DIM], ps_o[:, 1, :])
            nc.sync.dma_start(out=out_d[rc * 128:(rc + 1) * 128, :], in_=ot)

        if debug:
            nc.sync.dma_start(out=dbg_qT[:], in_=qT_sb)
            nc.sync.dma_start(out=dbg_kT[:], in_=kT2_sb)
            nc.sync.dma_start(out=dbg_vaug[:], in_=vaug_sb)
            nc.sync.dma_start(out=dbg_ctxT[:], in_=ctxT2_sb)

    nc.compile()
    return nc


def _get_nc():
    if "nc" not in _CACHE:
        _CACHE["nc"] = _build_nc()
    return _CACHE["nc"]


def _prep_inputs(x, wq, wk, wv, wo):
    GS = NH // NKV
    x2 = np.asarray(x, np.float32).reshape(S, DIN)
    xT = np.ascontiguousarray(x2.T)
    # chunk-major: xc[n, p, c, s'] = xT[c*128+p, n*QC+s']
    xc = np.ascontiguousarray(
        xT.reshape(8, 128, NQC, QC).transpose(2, 1, 0, 3).reshape(NQC, 128, 8 * QC)
    ).astype(BF16)
    # additive causal bias for the diagonal 128x128 block: 0 where q >= k,
    # -240 where masked (exp((s-240)/8) ~ 0)
    tri = np.where(
        np.arange(128)[None, :] >= np.arange(128)[:, None], 0.0, -240.0
    ).astype(BF16)
    in_maps = []
    for c in range(NCORES):
        h0 = 2 * c
        g = h0 // GS
        wq_c = np.asarray(wq, np.float32)[h0 * HD:(h0 + 2) * HD, :]
        wkv_c = np.concatenate(
            [
                np.asarray(wk, np.float32)[g * HD:(g + 1) * HD, :],
                np.asarray(wv, np.float32)[g * HD:(g + 1) * HD, :],
            ],
            axis=0,
        )
        woT_c = np.asarray(wo, np.float32)[:, h0 * HD:(h0 + 2) * HD].T

        def prearrange(wT):  # [1024, 128] -> [p, c*m] = [128, 1024]
            return np.ascontiguousarray(
                wT.reshape(8, 128, 128).transpose(1, 0, 2).reshape(128, 1024)
            )

        in_maps.append(
            {
                "xc": xc,
                "wqT": prearrange(np.ascontiguousarray(wq_c.T)).astype(BF16),
                "wkvT": prearrange(np.ascontiguousarray(wkv_c.T)).astype(BF16),
                "woT": np.ascontiguousarray(woT_c).astype(BF16),
                "trimask": tri,
            }
        )
    return in_maps


def _run(in_maps, trace=False):
    import sys
    if "/opt/trn_rl_repo" not in sys.path:
        sys.path.insert(0, "/opt/trn_rl_repo")
    from concourse.bass_utils import run_bass_kernel_spmd

    nc = _get_nc()
    res = run_bass_kernel_spmd(nc, in_maps, list(range(NCORES)), trace=trace)
    return res


def kernel(x, wq, wk, wv, wo):
    in_maps = _prep_inputs(x, wq, wk, wv, wo)
    res = _run(in_maps)
    parts = np.stack(
        [np.asarray(r["out"], np.float32) for r in res.results]
    )
    out = parts.sum(axis=0, dtype=np.float64).astype(np.float32)
    return out.reshape(1, S, DIM)


# revision 46
# speedup vs baseline: 1.0162x; 1.0071x over previous
"""GQA (16 q-heads / 4 KV groups, S=4096, D=1024, causal) on 8 TRN2 NeuronCores.

Sharding: tensor-parallel over query heads - 2 q-heads + their KV group per
core. wq/wk/wv column-sharded, wo row-sharded; the 8 partial outputs are
summed on the host (no device collectives needed).

Per-core program (bf16 matmuls, f32 PSUM), ACT(exp)-bound design, ~220us:
  x is loaded chunk-major: one DMA per 512-token chunk, fully contiguous
    (8KB per partition per chunk); wide warm-up matmuls unthrottle the PE
    (HAM) and a dummy exp preloads the ACT table while DMAs fly.
  Projections per chunk n (2 matmul/c-step, accumulated in psum_w):
    set1 lhsT=[wq_h0|wq_h1] -> qT_sb, set2 lhsT=[wk|wv] -> kT (duplicated to
    both partition halves) + vT.  All psum->sbuf copies on DVE.
    v normal layout via PE-transpose (identity matmul) of vT 128-col tiles
    -> vaug [128, kt, 65] (col 64 = ones for the denominator row).
  Attention per q-chunk (512 q x 2 heads), per k-tile of 128 keys:
    qk: two row-tiled concurrent matmuls -> ps [128, 2, 512] (2 psum banks),
    emitted three tiles ahead of ctx (and 3 tiles pre-issued per chunk) so
    the scalar engine's next exp input is always ready.
    causal diagonal: DVE adds a -240 bias on the masked 128x128 block of the
    psum scores BEFORE exp (exp((s-240)/8) ~ 0), so ctx follows exp with no
    extra mask step.
    exp(s/8): ONE ACT instr over both heads (strided on diag strips).
    ctx: 2 matmuls (lhsT=vaug [128 keys, 65], row 64 = ones -> denominators)
    accumulated in ctx_ps [65, 2, 512].
  Out-proj + projections for chunk n+2 + the deferred reciprocal chain are
    emitted as deadline-tagged "fillers", interleaved 1-2 per attention
    k-tile (proj(n) force-drained before attention(n)), so the PE queue
    never makes the scalar engine (the bottleneck: ~147us of exp) wait.
    The reciprocal chain pops AFTER the proj fillers (keeps the single-lane
    DVE ops away from the congested chunk boundary) and out-proj is
    debt-scheduled at least one full chunk after its normalize, so its
    matmuls never stall on the rb DMA round trip; chunk 6's out-proj lands
    in the tail, soaking up the last reciprocal's latency.
  Normalize: DVE reciprocal of denom row -> bf16 -> DRAM -> partition-
    broadcast DMA -> rb [128, 1024]; ctxT2 [128, S] scaled in place (gpsimd).
  Out-proj per 128-row block: 2 matmuls (contraction 128, both heads at
    once), one DVE copy psum -> bf16 -> DMA to out partial [S, DIM] bf16.
  Tail: last chunk's recips are PE-broadcast (no DRAM latency), normalize +
    out-proj pipelined per 128-row block on the freed scores psum ring with
    copies split scalar/DVE; warm matmuls bridge the reciprocal latency.
Softmax uses no max-subtraction: s/8 ~ N(0,1) -> exp safe in f32.
"""

import numpy as np
import ml_dtypes
from collections import deque

BF16 = ml_dtypes.bfloat16

S = 4096
DIN = 1024
DIM = 1024
NH, NKV, HD = 16, 4, 64
NCORES = 8
QC = 512          # q chunk width per head
NQC = S // QC     # 8
NKT = S // 128    # 32 k tiles

_CACHE = {}


def _build_nc(debug=False):
    import concourse.bass as bass
    import concourse.mybir as mybir
    import concourse.tile as tile
    from concourse import bacc
    from concourse.masks import make_identity
    from concourse.tile_rust import add_dep_helper
    from contextlib import ExitStack

    fp32 = mybir.dt.float32
    bf16 = mybir.dt.bfloat16
    Exp = mybir.ActivationFunctionType.Exp

    nc = bacc.Bacc()
    # chunk-major x: xc[n, p, c*QC+s'] = x[n*QC+s', c*128+p]
    xc_d = nc.dram_tensor("xc", [NQC, 128, 8 * QC], bf16, kind="ExternalInput")
    # host pre-arranged [p, c, m]: row-contiguous DMA (128 descriptors)
    wqT_d = nc.dram_tensor("wqT", [128, 8 * 128], bf16, kind="ExternalInput")
    wkvT_d = nc.dram_tensor("wkvT", [128, 8 * 128], bf16, kind="ExternalInput")
    woT_d = nc.dram_tensor("woT", [128, DIM], bf16, kind="ExternalInput")
    mask_d = nc.dram_tensor("trimask", [128, 128], bf16, kind="ExternalInput")
    out_d = nc.dram_tensor("out", [S, DIM], bf16, kind="ExternalOutput")
    rec_d = nc.dram_tensor("recips_scratch", [NQC, 2 * QC], bf16)
    if debug:
        dbg_qT = nc.dram_tensor("dbg_qT", [128, NQC, QC], bf16, kind="ExternalOutput")
        dbg_kT = nc.dram_tensor("dbg_kT", [128, S], bf16, kind="ExternalOutput")
        dbg_vaug = nc.dram_tensor("dbg_vaug", [128, NKT, 65], bf16, kind="ExternalOutput")
        dbg_ctxT = nc.dram_tensor("dbg_ctxT", [128, S], bf16, kind="ExternalOutput")
        dbg_cps = nc.dram_tensor("dbg_cps", [128, 2, QC], mybir.dt.float32, kind="ExternalOutput")
        dbg_rb = nc.dram_tensor("dbg_rb", [128, 2, QC], bf16, kind="ExternalOutput")

    with ExitStack() as ctx:
        tc = ctx.enter_context(tile.TileContext(nc))
        singles = ctx.enter_context(tc.tile_pool(name="singles", bufs=1))
        pt_pool = ctx.enter_context(tc.tile_pool(name="pt", bufs=5))
        small = ctx.enter_context(tc.tile_pool(name="small", bufs=2))
        ostage = ctx.enter_context(tc.tile_pool(name="ostage", bufs=3))
        # PSUM budget (16KB/partition): ps_s 2x4KB + ps_w 1x4KB + ctx 1x4KB
        psum_s = ctx.enter_context(tc.tile_pool(name="psum_s", bufs=2, space="PSUM"))
        psum_w = ctx.enter_context(tc.tile_pool(name="psum_w", bufs=1, space="PSUM"))
        psum_ctx = ctx.enter_context(
            tc.tile_pool(name="psum_ctx", bufs=1, space="PSUM")
        )

        # ---- persistent SBUF tensors ----
        xT_sb = singles.tile([128, NQC, 8, QC], bf16, tag="xT")
        wqT_sb = singles.tile([128, 8, 128], bf16, tag="wqT")
        wkvT_sb = singles.tile([128, 8, 128], bf16, tag="wkvT")
        woT_sb = singles.tile([128, DIM], bf16, tag="woT")
        mask_sb = singles.tile([128, 128], bf16, tag="mask")
        ident_sb = singles.tile([64, 64], bf16, tag="ident")
        ones_sb = singles.tile([1, 64], bf16, tag="ones")
        qT_sb = singles.tile([128, NQC, QC], bf16, tag="qT")
        kT2_sb = singles.tile([128, S], bf16, tag="kT2")
        vT_sb = singles.tile([64, S], bf16, tag="vT")
        vaug_sb = singles.tile([128, NKT, 65], bf16, tag="vaug")
        ctxT2_sb = singles.tile([128, S], bf16, tag="ctxT2")

        # ---- input DMAs (chunk 0 halves + weights first) ----
        nc.sync.dma_start(out=xT_sb[:, 0, 0:4], in_=xc_d[0, :, 0:4 * QC])
        nc.sync.dma_start(
            out=wqT_sb, in_=wqT_d[:].rearrange("p (c m) -> p c m", c=8)
        )
        nc.sync.dma_start(
            out=wkvT_sb, in_=wkvT_d[:].rearrange("p (c m) -> p c m", c=8)
        )
        nc.sync.dma_start(out=xT_sb[:, 0, 4:8], in_=xc_d[0, :, 4 * QC:8 * QC])
        nc.sync.dma_start(out=xT_sb[:, 1], in_=xc_d[1])
        nc.sync.dma_start(out=mask_sb, in_=mask_d[:])
        nc.sync.dma_start(out=woT_sb, in_=woT_d[:])
        for n in range(2, NQC):
            nc.sync.dma_start(out=xT_sb[:, n], in_=xc_d[n])
        nc.vector.memset(vaug_sb[:, :, 64:65], 1.0)
        nc.vector.memset(ones_sb, 1.0)
        nc.vector.memset(ctxT2_sb[0:64, 0:QC], 0.0)
        make_identity(nc, ident_sb)
        # preload the exp table set while DMAs are in flight
        warm_sb = singles.tile([64, 64], bf16, tag="warm")
        nc.scalar.activation(warm_sb[0:1, 0:1], ident_sb[0:1, 0:1], Exp)
        # keep the PE busy until x arrives so HAM unthrottles before proj(0)
        warm_ps = psum_w.tile([64, QC], fp32, tag="ps_w", name="warm_ps")
        for _ in range(12):
            nc.tensor.matmul(
                warm_ps[0:64, :], ident_sb, ctxT2_sb[0:64, 0:QC],
                start=True, stop=True,
            )

        # ---------- building blocks ----------
        def proj_pieces(n):
            """Return list of filler callables for chunk n's projections."""
            hold = {}

            def mm(c):
                def f():
                    if c == 0:
                        hold["ps"] = psum_w.tile([128, 2, QC], fp32, tag="ps_w", name="ps_proj")
                    ps = hold["ps"]
                    xs = xT_sb[:, n, c, :]
                    nc.tensor.matmul(
                        ps[:, 0, :], wqT_sb[:, c, :], xs,
                        start=(c == 0), stop=(c == 7),
                    )
                    nc.tensor.matmul(
                        ps[:, 1, :], wkvT_sb[:, c, :], xs,
                        start=(c == 0), stop=(c == 7),
                    )
                return f

            cs = slice(n * QC, (n + 1) * QC)

            def cp_v():
                nc.vector.tensor_copy(vT_sb[:, cs], hold["ps"][64:128, 1, :])

            def cp_kl():
                if n < 2:
                    nc.scalar.copy(kT2_sb[0:64, cs], hold["ps"][0:64, 1, :])
                else:
                    nc.vector.tensor_copy(kT2_sb[0:64, cs], hold["ps"][0:64, 1, :])

            def cp_kh():
                nc.vector.tensor_copy(kT2_sb[64:128, cs], hold["ps"][0:64, 1, :])

            def cp_q():
                nc.vector.tensor_copy(qT_sb[:, n, :], hold["ps"][:, 0, :])

            def tr(t):
                def f():
                    if t == 0:
                        hold["tr"] = psum_w.tile([128, 4, 256], bf16, tag="ps_w", name="ps_tr")
                    kt = 4 * n + t
                    nc.tensor.transpose(
                        hold["tr"][:, t, 0:64],
                        vT_sb[:, kt * 128:(kt + 1) * 128],
                        ident_sb,
                    )
                return f

            def cp_vaug():
                nc.vector.tensor_copy(
                    vaug_sb[:, 4 * n:4 * n + 4, 0:64], hold["tr"][:, :, 0:64]
                )

            return (
                [mm(c) for c in range(8)]
                + [cp_kl, cp_kh, cp_q, cp_v]
                + [tr(t) for t in range(4)]
                + [cp_vaug]
            )

        def outproj_pieces(qc, pool=None, tag="ps_w", split_copy=False):
            """Fillers: per 128-row block an mm step and a copy+store step."""
            pool = pool or psum_w
            pieces = []
            hold = {}
            for rc in range(4 * qc, 4 * qc + 4):
                def mm(rc=rc):
                    ps_o = pool.tile([128, 2, QC], fp32, tag=tag, name="ps_o")
                    hold["ps"] = ps_o
                    lh = ctxT2_sb[:, rc * 128:(rc + 1) * 128]
                    for e in range(2):
                        nc.tensor.matmul(
                            ps_o[:, e, :], lh, woT_sb[:, e * QC:(e + 1) * QC],
                            start=True, stop=True,
                        )

                def st(rc=rc):
                    ot = ostage.tile([128, DIM], bf16, tag="ot")
                    if split_copy:
                        nc.scalar.copy(ot[:, 0:QC], hold["ps"][:, 0, :])
                        nc.vector.tensor_copy(ot[:, QC:DIM], hold["ps"][:, 1, :])
                    else:
                        nc.vector.tensor_copy(ot, hold["ps"][:, :, :])
                    nc.sync.dma_start(
                        out=out_d[rc * 128:(rc + 1) * 128, :], in_=ot
                    )

                pieces += [mm, st]
            return pieces

        def normalize(qc, rb):
            cs = slice(qc * QC, (qc + 1) * QC)
            nc.gpsimd.tensor_mul(
                ctxT2_sb[0:64, cs], ctxT2_sb[0:64, cs], rb[0:64, 0, :]
            )
            nc.gpsimd.tensor_mul(
                ctxT2_sb[64:128, cs], ctxT2_sb[64:128, cs], rb[64:128, 1, :]
            )

        fillers = deque()

        def pop_fillers(k, warm=False):
            for _ in range(k):
                if fillers:
                    fillers.popleft()[1]()
                elif warm:
                    return

        def attention(qc):
            nkt = 4 * qc + 4

            def qk_exp(kt):
                r = kt - 4 * qc
                off = 128 * r if r >= 1 else 0
                ps = psum_s.tile([128, 2, QC], fp32, tag="ps_s", name="ps")
                pt = pt_pool.tile([128, 2, QC], bf16, tag="pt", name="pt")
                ktl = kT2_sb[0:64, kt * 128:(kt + 1) * 128]
                kth = kT2_sb[64:128, kt * 128:(kt + 1) * 128]
                nc.tensor.matmul(
                    ps[:, 0, off:QC], ktl, qT_sb[0:64, qc, off:QC],
                    start=True, stop=True,
                )
                nc.tensor.matmul(
                    ps[:, 1, off:QC], kth, qT_sb[64:128, qc, off:QC],
                    start=True, stop=True, tile_position=(64, 0),
                )
                if r >= 0:  # causal diagonal block: add -240 bias pre-exp
                    for h in range(2):
                        nc.vector.tensor_add(
                            ps[:, h, off:off + 128],
                            ps[:, h, off:off + 128],
                            mask_sb,
                        )
                nc.scalar.activation(
                    pt[:, :, off:QC], ps[:, :, off:QC], Exp, scale=0.125
                )
                return pt

            # scores always run two tiles ahead of ctx so the scalar
            # engine's next exp input is ready before it finishes the current
            pts = {kt: qk_exp(kt) for kt in range(min(3, nkt))}
            ctx_ps = psum_ctx.tile([128, 2, QC], fp32, tag="ps_ctx")
            for kt in range(nkt):
                r = kt - 4 * qc
                off = 128 * r if r >= 1 else 0
                if kt + 3 < nkt:
                    pts[kt + 3] = qk_exp(kt + 3)
                pt = pts.pop(kt)
                va = vaug_sb[:, kt, :]
                nc.tensor.matmul(
                    ctx_ps[0:65, 0, off:QC], va, pt[:, 0, off:QC],
                    start=(kt == 0), stop=(kt == nkt - 1),
                )
                nc.tensor.matmul(
                    ctx_ps[0:65, 1, off:QC], va, pt[:, 1, off:QC],
                    start=(kt == 0), stop=(kt == nkt - 1),
                )
                pop_fillers(2 if (kt < 6 or len(fillers) > 32) else 1,
                            warm=(qc >= 5))
            return ctx_ps

        def finish_ctx(qc, ctx_ps, last=False):
            """Copy raw ctx to stacked sbuf and stage the denominator row;
            the reciprocal chain is deferred into the next chunk's fillers."""
            cs = slice(qc * QC, (qc + 1) * QC)
            den = small.tile([1, 2, QC], fp32, tag="den")
            nc.vector.tensor_copy(den, ctx_ps[64:65, :, :])
            if last:  # scalar engine is idle in the tail
                nc.scalar.copy(ctxT2_sb[0:64, cs], ctx_ps[0:64, 0, :])
                nc.scalar.copy(ctxT2_sb[64:128, cs], ctx_ps[0:64, 1, :])
            else:
                nc.vector.tensor_copy(ctxT2_sb[0:64, cs], ctx_ps[0:64, 0, :])
                nc.vector.tensor_copy(ctxT2_sb[64:128, cs], ctx_ps[0:64, 1, :])
            return den

        def make_rb(den, last=False, row=0):
            """Reciprocal of the staged denominators + partition broadcast
            (DRAM round trip); for the tail, return recb for a PE broadcast."""
            nc.vector.reciprocal_approx_fast(den, den)
            recb = small.tile([1, 2, QC], bf16, tag="recb")
            nc.vector.tensor_copy(recb, den)
            if last:
                return recb
            rb = small.tile([128, 2, QC], bf16, tag="rb")
            w1 = nc.sync.dma_start(out=rec_d[row:row + 1, :], in_=recb)
            srcp = rec_d[row, :]
            r1 = nc.sync.dma_start(
                out=rb,
                in_=bass.AP(tensor=srcp.tensor, offset=srcp.offset,
                            ap=[[0, 128]] + list(srcp.ap)),
            )
            add_dep_helper(r1.ins, w1.ins, reason="recips dram RAW")
            return rb

        # ---- main pipeline ----
        # proj(0) runs eagerly; proj(1) drains into attention(0);
        # proj(n+2) + outproj(n-1) interleave into attention(n), a couple
        # of filler pieces per k-tile.
        for f in proj_pieces(0):
            f()
        fillers.extend((0, f) for f in proj_pieces(1))
        prev = None  # (qc, den) awaiting recip+normalize+outproj
        oproj_debt = deque()  # chunks normalized but not yet projected out
        hold_rb = {}
        NODL = 99  # no deadline

        def force_due(qc):
            due = -1
            for i, (t, _) in enumerate(fillers):
                if t <= qc:
                    due = i
            for _ in range(due + 1):
                fillers.popleft()[1]()

        for qc in range(NQC):
            if qc + 2 < NQC:
                # proj(qc+2) must fully drain by the end of attention(qc+1)
                fillers.extend((qc + 1, f) for f in proj_pieces(qc + 2))
            if prev is not None:
                pq, den = prev
                oproj_debt.append(pq)

                def mk(pq=pq, den=den):
                    hold_rb[pq] = make_rb(den, row=pq)
                fillers.append((NODL, mk))
                fillers.append(
                    (NODL, lambda pq=pq: normalize(pq, hold_rb.pop(pq)))
                )
            quota = 1 if qc < 6 else 2
            for _ in range(quota):
                # only chunks normalized in an EARLIER iteration (no stall
                # on the fresh rb round trip)
                if oproj_debt and (prev is None or oproj_debt[0] != prev[0]):
                    fillers.extend(
                        (NODL, f) for f in outproj_pieces(oproj_debt.popleft())
                    )
            ctx_ps = attention(qc)
            force_due(qc)  # next chunk's proj ahead of the DVE den staging
            den = finish_ctx(qc, ctx_ps, last=(qc == NQC - 1))
            prev = (qc, den)
        while fillers:
            fillers.popleft()[1]()
        for pq2 in oproj_debt:  # any outproj still owed (e.g. chunk 6)
            for f in outproj_pieces(pq2):
                f()
        # tail: keep the PE warm while the reciprocal chain runs, then
        # PE-broadcast the recips (no DRAM latency) and pipeline
        # normalize+outproj per 128-row block on the free scores psum ring
        pq, den = prev
        recb = make_rb(den, last=True)
        warm2 = psum_s.tile([64, QC], fp32, tag="ps_s", name="warm2")
        for _ in range(14):
            nc.tensor.matmul(warm2[0:64, :], ident_sb, ctxT2_sb[0:64, 0:QC],
                             start=True, stop=True)
        bps = psum_w.tile([128, 2, QC], fp32, tag="ps_w", name="bps")
        nc.tensor.matmul(bps[0:64, 0, :], ones_sb, recb[0:1, 0, :],
                         start=True, stop=True)
        nc.tensor.matmul(bps[0:64, 1, :], ones_sb, recb[0:1, 1, :],
                         start=True, stop=True)
        rbs = small.tile([128, 2, QC], bf16, tag="rb")
        for i, rc in enumerate(range(4 * pq, 4 * pq + 4)):
            cb = slice(rc * 128, (rc + 1) * 128)
            qb = slice(i * 128, (i + 1) * 128)
            nc.vector.tensor_copy(rbs[0:64, 0, qb], bps[0:64, 0, qb])
            nc.vector.tensor_copy(rbs[64:128, 1, qb], bps[0:64, 1, qb])
            nc.vector.tensor_mul(
                ctxT2_sb[0:64, cb], ctxT2_sb[0:64, cb], rbs[0:64, 0, qb]
            )
            nc.vector.tensor_mul(
                ctxT2_sb[64:128, cb], ctxT2_sb[64:128, cb], rbs[64:128, 1, qb]
            )
            ps_o = psum_s.tile([128, 2, QC], fp32, tag="ps_s", name="ps_ot")
            lh = ctxT2_sb[:, cb]
            for e in range(2):
                nc.tensor.matmul(
                    ps_o[:, e, :], lh, woT_sb[:, e * QC:(e + 1) * QC],
                    start=True, stop=True,
                )
            ot = ostage.tile([128, DIM], bf16, tag="ot")
            nc.scalar.copy(ot[:, 0:QC], ps_o[:, 0, :])
            nc.vector.tensor_copy(ot[:, QC:DIM], ps_o[:, 1, :])
            nc.sync.dma_start(out=out_d[rc * 128:(rc + 1) * 128, :], in_=ot)

        if debug:
            nc.sync.dma_start(out=dbg_qT[:], in_=qT_sb)
            nc.sync.dma_start(out=dbg_kT[:], in_=kT2_sb)
            nc.sync.dma_start(out=dbg_vaug[:], in_=vaug_sb)
            nc.sync.dma_start(out=dbg_ctxT[:], in_=ctxT2_sb)

    nc.compile()
    return nc


def _get_nc():
    if "nc" not in _CACHE:
        _CACHE["nc"] = _build_nc()
    return _CACHE["nc"]


def _prep_inputs(x, wq, wk, wv, wo):
    GS = NH // NKV
    x2 = np.asarray(x, np.float32).reshape(S, DIN)
    xT = np.ascontiguousarray(x2.T)
    # chunk-major: xc[n, p, c, s'] = xT[c*128+p, n*QC+s']
    xc = np.ascontiguousarray(
        xT.reshape(8, 128, NQC, QC).transpose(2, 1, 0, 3).reshape(NQC, 128, 8 * QC)
    ).astype(BF16)
    # additive causal bias for the diagonal 128x128 block: 0 where q >= k,
    # -240 where masked (exp((s-240)/8) ~ 0)
    tri = np.where(
        np.arange(128)[None, :] >= np.arange(128)[:, None], 0.0, -240.0
    ).astype(BF16)
    in_maps = []
    for c in range(NCORES):
        h0 = 2 * c
        g = h0 // GS
        wq_c = np.asarray(wq, np.float32)[h0 * HD:(h0 + 2) * HD, :]
        wkv_c = np.concatenate(
            [
                np.asarray(wk, np.float32)[g * HD:(g + 1) * HD, :],
                np.asarray(wv, np.float32)[g * HD:(g + 1) * HD, :],
            ],
            axis=0,
        )
        woT_c = np.asarray(wo, np.float32)[:, h0 * HD:(h0 + 2) * HD].T

        def prearrange(wT):  # [1024, 128] -> [p, c*m] = [128, 1024]
            return np.ascontiguousarray(
                wT.reshape(8, 128, 128).transpose(1, 0, 2).reshape(128, 1024)
            )

        in_maps.append(
            {
                "xc": xc,
                "wqT": prearrange(np.ascontiguousarray(wq_c.T)).astype(BF16),
                "wkvT": prearrange(np.ascontiguousarray(wkv_c.T)).astype(BF16),
                "woT": np.ascontiguousarray(woT_c).astype(BF16),
                "trimask": tri,
            }
        )
    return in_maps


def _run(in_maps, trace=False):
    import sys
    if "/opt/trn_rl_repo" not in sys.path:
        sys.path.insert(0, "/opt/trn_rl_repo")
    from concourse.bass_utils import run_bass_kernel_spmd

    nc = _get_nc()
    res = run_bass_kernel_spmd(nc, in_maps, list(range(NCORES)), trace=trace)
    return res


def kernel(x, wq, wk, wv, wo):
    in_maps = _prep_inputs(x, wq, wk, wv, wo)
    res = _run(in_maps)
    parts = np.stack(
        [np.asarray(r["out"], np.float32) for r in res.results]
    )
    out = parts.sum(axis=0, dtype=np.float64).astype(np.float32)
    return out.reshape(1, S, DIM)
